# revision 22
# baseline (speedup 1.0000x reference)
"""Distributed Trainium2 kernel for LoRA multi-head causal attention.

Problem: out = (softmax(causal(RoPE(x@wq'.T) @ RoPE(x@wk'.T)^T / sqrt(dh)))
               @ (x@wv'.T)) @ wo.T
where wq' = wq + LORA_SCALE * lq_up @ lq_down (LoRA folded into the dense
weights on the HOST -- exact linear algebra, so the device never sees the
low-rank path).
Shapes: B=4, S=2048, D=2048, H=16 heads, DH=128, fp32 I/O.

Sharding (8 NeuronCores): 2-D grid of (batch b = core//2) x (head-group
j = core%2, 8 heads each).  Each core computes its batch's attention for its
8 heads plus the partial o_proj for those heads' feature rows; partials are
summed with a pairwise ReduceScatter (cores 2b, 2b+1), each core emitting
half the sequence rows of batch b.  The host concatenates the halves.

Device-side plan per core (all matmuls bf16 with fp32 PSUM accumulation):
  A. every operand arrives from the HOST pre-transposed (contraction dim
     leading), pair-deinterleaved where RoPE needs it, and pre-cast to
     bf16 -- phase A is pure DMA.
  B. V = x@wv'.T in [s, m] layout, SBUF-resident; then per head Q^T/K^T
     [dh, s] with a fused RoPE eviction writing DIRECTLY into persistent
     SBUF tiles (no DRAM bounce -- Q^T and K^T stay resident through the
     attention phase, 8 MiB).
  C. Attention, query-chunk outer / head inner, causal blocks only:
     scores^T = K_tile @ Q^T (one transposed [128,128] mask tile on the
     diagonal), ACT exp straight out of PSUM into bf16 P^T.  The softmax
     denominator is accumulated on the DVE (acc += P^T tile per key tile)
     and reduced over partitions with ONE ones-column matmul per
     (head, chunk) -- this removes ~90us of [128,1,512] matmuls from the
     in-order PE queue.  P@V accumulates over key tiles in PSUM.
  D. o_proj from the attention outputs' transposed layout; per-head 1/l
     normalization via exp(-ln(l)) on ACT + a ones-broadcast matmul.
  E. Pairwise ReduceScatter of the partial bf16 output per 128-row tile,
     written directly into the kernel output buffer; the collective for
     tile st overlaps o_proj of st+1.
"""

import sys

for _p in ("/opt/trn_rl_repo", "/root/.axon_site/_ro/trn_rl_repo"):
    if _p not in sys.path:
        sys.path.append(_p)

import numpy as np

import concourse.bass as bass
import concourse.mybir as mybir
import concourse.tile as tile
from concourse.masks import make_identity

F32 = mybir.dt.float32
BF16 = mybir.dt.bfloat16
AF = mybir.ActivationFunctionType

B, S, D, H = 4, 2048, 2048, 16
DH = 128
H_LOC = 8           # heads per core
M = H_LOC * DH      # 1024: local qkv feature dim
SCALE = 1.0 / float(np.sqrt(DH))
LORA_SCALE = 32.0 / 16.0
N_CORES = 8
SC = 512            # q/s chunk size
NQC = S // SC       # 4
NDT = D // 128      # 16 contraction tiles
NMT = M // 128      # 8 local m tiles (= local heads)
NST = S // 128      # 16 sequence tiles

# ---------------------------------------------------------------------------
# Walrus in this container rejects instructions carrying more than one sync
# wait ("Too many sync wait commands").  After Tile scheduling, move excess
# semaphore waits onto same-engine nop instructions inserted immediately
# before the over-subscribed instruction (same sequencer, same order, so
# semantics are unchanged).
# ---------------------------------------------------------------------------


def _split_sync_waits(nc, limit=1):
    for bb in nc.main_func.blocks:
        out = []
        changed = False
        for inst in bb.instructions:
            si = inst.sync_info
            waits = list(si.on_wait) if si is not None else []
            if len(waits) > limit:
                changed = True
                extra, keep = waits[:-limit], waits[-limit:]
                for i in range(0, len(extra), limit):
                    n = mybir.InstNoOp(name=nc.get_next_instruction_name())
                    n.engine = inst.engine
                    n.sync_info = mybir.SyncInfo(
                        on_wait=extra[i : i + limit], on_update=[]
                    )
                    nc.register_instruction(n)
                    out.append(n)
                inst.sync_info = mybir.SyncInfo(
                    on_wait=keep, on_update=list(si.on_update)
                )
            out.append(inst)
        if changed:
            bb.instructions = out


class FixedTileContext(tile.TileContext):
    def __exit__(self, exc_type, exc_value, traceback):
        result = super().__exit__(exc_type, exc_value, traceback)
        if exc_type is None:
            _split_sync_waits(self.nc)
        return result


# ---------------------------------------------------------------------------
# Kernel builder (one SPMD graph, identical on all 8 cores)
# ---------------------------------------------------------------------------


def build_nc():
    nc = bass.Bass(target_bir_lowering=False)

    # All weight/activation operands arrive pre-transposed (and where
    # needed pair-deinterleaved) from the host, pre-cast to bf16, with
    # LoRA already folded into the dense weights.
    xT_ext = nc.declare_dram_parameter("xT", [D, S], BF16, isOutput=False)
    wqT_ext = nc.declare_dram_parameter("wqT", [D, M], BF16, isOutput=False)
    wkT_ext = nc.declare_dram_parameter("wkT", [D, M], BF16, isOutput=False)
    wvT_ext = nc.declare_dram_parameter("wvT", [D, M], BF16, isOutput=False)
    woT_ext = nc.declare_dram_parameter("woT", [M, D], BF16, isOutput=False)
    tabC_ext = nc.declare_dram_parameter("tabC", [128, S], BF16, isOutput=False)
    tabS_ext = nc.declare_dram_parameter("tabS", [128, S], BF16, isOutput=False)
    triTs_ext = nc.declare_dram_parameter("triTs", [128, 128], BF16, isOutput=False)
    # bf16 output: the host upcasts to f32 in assemble()
    out_ext = nc.declare_dram_parameter("out", [S // 2, D], BF16, isOutput=True)

    out_partial = nc.dram_tensor("out_partial", [S, D], BF16)
    out_rs = nc.dram_tensor("out_rs", [S // 2, D], BF16)

    with FixedTileContext(nc) as tc:
        with tc.tile_pool(name="const", bufs=1) as const:
            id_bf16 = const.tile([128, 128], BF16, tag="idh")
            make_identity(nc, id_bf16)
            ones_sq = const.tile([128, 128], BF16, tag="ones_sq")
            nc.vector.memset(ones_sq[:, :], 1.0)
            triT_h = const.tile([128, 128], BF16, tag="triT_h")
            # first B2 weight tile, preloaded on the idle scalar queue in
            # phase A: the wtt pool reuses wtv's SBUF zone, so its own DMA
            # loads can't start until B1's last matmul releases it.
            wtb0 = const.tile([128, NDT, 128], BF16, tag="wtb0")

            _phases(nc, tc, locals())
    return nc


def _attn_epilogue(nc, g, sc_ps, linv_pool, at_pool, AT, qc, h, po, acc):
    # Deferred one head so the PE never waits on the DVE accumulation
    # chain's tail.  The all-ones [128,128] stationary replicates the
    # partition sum onto ALL output partitions, so 1/l is one full-rate
    # exact DVE reciprocal -- no ACT chain, no broadcast step.
    pb = sc_ps.tile([128, SC], F32, tag="ps", name=f"pb{h}_{qc}")
    nc.tensor.matmul(
        pb[:, :], g["ones_sq"][:, :], acc[:, :], start=True, stop=True
    )
    lb = linv_pool.tile([128, SC], F32, tag="lb", name=f"lb{h}_{qc}")
    nc.vector.reciprocal(lb[:, :], pb[:, :])
    at_ = at_pool.tile([128, SC], BF16, tag=f"aT{h}", name=f"aT{h}_{qc}")
    nc.vector.tensor_mul(at_[:, :], po[:, :], lb[:, :])
    AT[h] = at_


def _phases(nc, tc, t):
    g = t
    xT_ext = g["xT_ext"]
    out_ext, out_partial, out_rs = g["out_ext"], g["out_partial"], g["out_rs"]
    id_bf16 = g["id_bf16"]
    triT_h = g["triT_h"]

    vs_ctx = tc.tile_pool(name="vs", bufs=1)
    vs_pool = vs_ctx.__enter__()
    VS = [vs_pool.tile([128, M], BF16, tag=f"v{st}", name=f"v{st}") for st in range(NST)]
    # Q^T/K^T stay SBUF-resident through the attention phase (8 MiB).
    # Created below ab on the pool stack so they survive ab's release.
    qkres_ctx = tc.tile_pool(name="qkres", bufs=1)
    qkres_pool = qkres_ctx.__enter__()
    QT = [qkres_pool.tile([128, S], BF16, tag=f"qres{h}", name=f"qres{h}")
          for h in range(H_LOC)]
    KT = [qkres_pool.tile([128, S], BF16, tag=f"kres{h}", name=f"kres{h}")
          for h in range(H_LOC)]
    wts_o = {}

    with tc.tile_pool(name="ab", bufs=1) as ab_pool:
        # rope tables live through phase B only.  Layout: cos duplicated on
        # rows 0:64 and 64:128; same for sin (keeps every rope product
        # partition-aligned).  1/sqrt(dh) is folded into the exp activation.
        tabkC = ab_pool.tile([128, S], BF16, tag="tabkC")
        tabkS = ab_pool.tile([128, S], BF16, tag="tabkS")
        XT = [ab_pool.tile([128, S], BF16, tag=f"xT{dt}", name=f"xT{dt}") for dt in range(NDT)]

        wtv_ctx = tc.tile_pool(name="wtv", bufs=16)
        wtv_pool = wtv_ctx.__enter__()
        # ------------------------------------------------------------------
        # Phase A: pure DMA loads.  wv on the gpsimd SWDGE ring, X^T on the
        # sync HWDGE queue in parallel, small constants on the scalar queue.
        # ------------------------------------------------------------------
        wts = [
            wtv_pool.tile([128, M], BF16, tag="wtv", name=f"wvT{dt}")
            for dt in range(NDT)
        ]
        for dt in range(NDT):
            nc.gpsimd.dma_start(
                out=wts[dt][:, :],
                in_=g["wvT_ext"][dt * 128 : (dt + 1) * 128, :],
            )
        for dt in range(NDT):
            nc.sync.dma_start(
                out=XT[dt][:, 0:SC],
                in_=xT_ext[dt * 128 : (dt + 1) * 128, 0:SC],
            )

        nc.scalar.dma_start(out=triT_h[:, :], in_=g["triTs_ext"][:, :])
        nc.scalar.dma_start(out=tabkC[:, :], in_=g["tabC_ext"][:, :])
        nc.scalar.dma_start(out=tabkS[:, :], in_=g["tabS_ext"][:, :])
        rr_q0 = g["wqT_ext"].rearrange("(dt p) m -> p dt m", p=128)
        nc.scalar.dma_start(out=g["wtb0"][:, :, :], in_=rr_q0[:, :, 0:128])

        for dt in range(NDT):
            nc.sync.dma_start(
                out=XT[dt][:, SC : 2 * SC],
                in_=xT_ext[dt * 128 : (dt + 1) * 128, SC : 2 * SC],
            )
        for dt in range(NDT):
            nc.sync.dma_start(
                out=XT[dt][:, 2 * SC : S],
                in_=xT_ext[dt * 128 : (dt + 1) * 128, 2 * SC : S],
            )

        # ------------------------------------------------------------------
        # Phase B1: V projection (SBUF-resident V).  One PSUM pool serves
        # both B1's pv and B2's pq tiles (same shape) so there is no
        # pool-boundary drain between the phases.
        # ------------------------------------------------------------------
        proj_ctx = tc.tile_pool(name="proj_ps", bufs=3, space="PSUM")
        proj_ps1 = proj_ctx.__enter__()
        for st in range(NST):
            pv = proj_ps1.tile([128, M], F32, tag="pv")
            ssl = slice(st * 128, (st + 1) * 128)
            for sub in range(2):
                psl = slice(sub * SC, (sub + 1) * SC)
                for dt in range(NDT):
                    nc.tensor.matmul(
                        pv[:, psl], XT[dt][:, ssl], wts[dt][:, psl],
                        start=(dt == 0), stop=(dt == NDT - 1),
                    )
            nc.vector.tensor_copy(VS[st][:, :], pv[:, :])
        wtv_ctx.__exit__(None, None, None)

        # ------------------------------------------------------------------
        # Phase B2: Q^T / K^T, head outer (q then k per head so attention on
        # head h can begin while head h+1 projects), rope eviction split
        # across ACT (psum spill), DVE, and GpSimd, writing straight into
        # the resident SBUF tiles.
        # ------------------------------------------------------------------
        HS = S // 2  # 1024: half the sequence per psum tile
        proj_ps2 = proj_ps1
        with tc.tile_pool(name="wtt", bufs=3) as wtt_pool, \
             tc.tile_pool(name="rope_tmp", bufs=2) as rope_tmp:
            rr_q = g["wqT_ext"].rearrange("(dt p) m -> p dt m", p=128)
            rr_k = g["wkT_ext"].rearrange("(dt p) m -> p dt m", p=128)
            for mt in range(NMT):
                for which, rr, outd, tabC, tabS in (
                    ("q", rr_q, QT, tabkC, tabkS),
                    ("k", rr_k, KT, tabkC, tabkS),
                ):
                    if mt == 0 and which == "q":
                        wtb = g["wtb0"]  # preloaded in phase A
                    else:
                        wtb = wtt_pool.tile([128, NDT, 128], BF16, tag="wtb")
                        nc.gpsimd.dma_start(
                            out=wtb[:, :, :],
                            in_=rr[:, :, mt * 128 : (mt + 1) * 128],
                        )
                    for half in range(2):
                        sl = slice(half * HS, (half + 1) * HS)
                        pq = proj_ps2.tile([128, HS], F32, tag="pv")
                        for sub in range(2):
                            psl = slice(sub * SC, (sub + 1) * SC)
                            xsl = slice(half * HS + sub * SC, half * HS + (sub + 1) * SC)
                            for dt in range(NDT):
                                nc.tensor.matmul(
                                    pq[:, psl], wtb[:, dt, :], XT[dt][:, xsl],
                                    start=(dt == 0), stop=(dt == NDT - 1),
                                )
                        # rope on deinterleaved pairs: rows 0:64 = a (even
                        # lanes), 64:128 = b (odd lanes); tab rows 0:64 cos,
                        # 64:128 sin.
                        if mt < 6:
                            qsb = rope_tmp.tile([128, HS], BF16, tag="qsb")
                            nc.scalar.copy(qsb[:, :], pq[:, :])
                        t1 = rope_tmp.tile([128, HS], BF16, tag="t1")
                        t2 = rope_tmp.tile([128, HS], BF16, tag="t2")
                        # t1 rows 0:64 = a*cos, rows 64:128 = b*cos.  GpSimd
                        # helps mid-phase; near the phase tail its queue
                        # backlog would delay the final evictions, so the
                        # last heads use DVE instead.
                        if mt < 6:
                            nc.gpsimd.tensor_mul(t1[:, :], qsb[:, :], tabC[:, sl])
                        else:
                            nc.vector.tensor_mul(t1[:, :], pq[:, :], tabC[:, sl])
                        # cross-half products from psum on DVE:
                        # t2 rows 0:64 = b*sin, rows 64:128 = a*sin
                        nc.vector.tensor_mul(t2[0:64, :], pq[64:128, :], tabS[0:64, sl])
                        nc.vector.tensor_mul(t2[64:128, :], pq[0:64, :], tabS[64:128, sl])
                        # even out = a*cos - b*sin ; odd out = a*sin + b*cos
                        # -- written directly into the resident tile
                        nc.vector.tensor_sub(outd[mt][0:64, sl], t1[0:64, :], t2[0:64, :])
                        nc.vector.tensor_add(outd[mt][64:128, sl], t2[64:128, :], t1[64:128, :])

        proj_ctx.__exit__(None, None, None)

    # ----------------------------------------------------------------------
    # Phases C+D interleaved, query-chunk outer: attention for all heads of
    # one 512-query chunk, then that chunk's o_proj rows and its pairwise
    # ReduceScatter -- the collective for chunk c overlaps attention of
    # chunk c+1.
    # ----------------------------------------------------------------------
    wo_ctx = tc.tile_pool(name="wo_t", bufs=8)
    wo_pool = wo_ctx.__enter__()
    with tc.tile_pool(name="at", bufs=2) as at_pool, \
         tc.tile_pool(name="pt", bufs=8) as pt_pool, \
         tc.tile_pool(name="acc", bufs=3) as acc_pool, \
         tc.tile_pool(name="o_sb", bufs=4) as o_sb, \
         tc.tile_pool(name="sc_ps", bufs=4, space="PSUM") as sc_ps, \
         tc.tile_pool(name="pv_ps", bufs=2, space="PSUM") as pv_ps, \
         tc.tile_pool(name="linv", bufs=4) as linv_pool:

        # o_proj weights stream in on the gpsimd ring while attention of
        # chunk 0 runs (first use is chunk 0's o_proj, ~40us in); loading
        # them here keeps them out of phase B's SBUF high-water mark.
        for mt in range(NMT):
            wob = wo_pool.tile([128, D], BF16, tag="wot", name=f"woT{mt}")
            nc.gpsimd.dma_start(
                out=wob[:, :],
                in_=g["woT_ext"][mt * 128 : (mt + 1) * 128, :],
            )
            wts_o[mt] = wob

        # chunk 1 first: its 8x8-key attention gives the o_proj weight DMAs
        # (which can only start once phase B's SBUF zone is released) enough
        # runway before the first o_proj needs them.  Chunk 3 stays last.
        for qc in (1, 0, 2, 3):
            nk = 4 * (qc + 1)
            AT = {}
            pending = []
            for h in range(H_LOC):
                po = pv_ps.tile([128, SC], F32, tag="po")
                acc = acc_pool.tile([128, SC], BF16, tag="acc", name=f"acc{h}_{qc}")
                for ki in range(nk):
                    q_off = max(0, ki * 128 - qc * SC)
                    csl = slice(q_off, SC)
                    ps_ = sc_ps.tile([128, SC], F32, tag="ps")
                    diag = ki * 128 >= qc * SC
                    nc.tensor.matmul(
                        ps_[:, csl],
                        KT[h][:, ki * 128 : (ki + 1) * 128],
                        QT[h][:, qc * SC + q_off : (qc + 1) * SC],
                        start=True, stop=not diag,
                    )
                    if diag:  # causal mask accumulated on the PE itself
                        nc.tensor.matmul(
                            ps_[:, q_off : q_off + 128],
                            id_bf16[:, :],
                            triT_h[:, :],
                            start=False, stop=True,
                        )
                    pt_ = pt_pool.tile([128, SC], BF16, tag="pt")
                    nc.scalar.activation(pt_[:, csl], ps_[:, csl], AF.Exp, scale=SCALE)
                    # softmax denominator: accumulate P^T tiles on the DVE
                    # instead of a per-ki ones-matmul on the in-order PE
                    # queue; the partition reduce happens in the (deferred)
                    # epilogue.
                    if ki == 0:
                        nc.vector.tensor_copy(acc[:, :], pt_[:, :])
                    else:
                        nc.vector.tensor_add(acc[:, csl], acc[:, csl], pt_[:, csl])
                    nc.tensor.matmul(
                        po[:, csl],
                        VS[ki][:, h * 128 : (h + 1) * 128],
                        pt_[:, csl],
                        start=(ki == 0), stop=(ki == nk - 1),
                    )
                pending.append((h, po, acc))
                if len(pending) > 1:
                    _attn_epilogue(nc, g, sc_ps, linv_pool, at_pool, AT, qc, *pending.pop(0))

            while pending:
                _attn_epilogue(nc, g, sc_ps, linv_pool, at_pool, AT, qc, *pending.pop(0))

            # o_proj rows of this chunk; ReduceScatter per 128-row st tile so
            # the collective for st overlaps o_proj of st+1.  For the last
            # chunk, st 12 is processed LAST so the big 384-row collective
            # (rows 13:16) overlaps st 12's o_proj and only a 128-row
            # collective plus a 128-row output copy are exposed at the tail.
            st_order = [13, 14, 15, 12] if qc == NQC - 1 else \
                list(range(qc * 4, (qc + 1) * 4))
            for idx, st in enumerate(st_order):
                ssl = slice((st % 4) * 128, (st % 4 + 1) * 128)
                if idx == 0:
                    # first st of the chunk: interleave the four PSUM
                    # chains -- heads 0-6 for every oc first, head 7 last,
                    # so the last head's epilogue finishes while the PE
                    # works instead of stalling the first chain
                    posums = []
                    for oc in range(NQC):
                        posum = sc_ps.tile(
                            [128, SC], F32, tag="ps", name=f"posum{st}_{oc}"
                        )
                        for mt in range(NMT - 1):
                            nc.tensor.matmul(
                                posum[:, :],
                                AT[mt][:, ssl],
                                wts_o[mt][:, oc * SC : (oc + 1) * SC],
                                start=(mt == 0), stop=False,
                            )
                        posums.append(posum)
                    for oc in range(NQC):
                        nc.tensor.matmul(
                            posums[oc][:, :],
                            AT[NMT - 1][:, ssl],
                            wts_o[NMT - 1][:, oc * SC : (oc + 1) * SC],
                            start=False, stop=True,
                        )
                        ot = o_sb.tile([128, SC], BF16, tag="ot")
                        nc.vector.tensor_copy(ot[:, :], posums[oc][:, :])
                        nc.sync.dma_start(
                            out=out_partial[
                                st * 128 : (st + 1) * 128, oc * SC : (oc + 1) * SC
                            ],
                            in_=ot[:, :],
                        )
                else:
                    for oc in range(NQC):
                        posum = sc_ps.tile(
                            [128, SC], F32, tag="ps", name=f"posum{st}_{oc}"
                        )
                        for mt in range(NMT):
                            nc.tensor.matmul(
                                posum[:, :],
                                AT[mt][:, ssl],
                                wts_o[mt][:, oc * SC : (oc + 1) * SC],
                                start=(mt == 0), stop=(mt == NMT - 1),
                            )
                        ot = o_sb.tile([128, SC], BF16, tag="ot")
                        nc.vector.tensor_copy(ot[:, :], posum[:, :])
                        nc.sync.dma_start(
                            out=out_partial[
                                st * 128 : (st + 1) * 128, oc * SC : (oc + 1) * SC
                            ],
                            in_=ot[:, :],
                        )
                if qc == NQC - 1:
                    if st == 15:
                        # rows 13:16 collective fires while st 12's o_proj
                        # still runs
                        r0, r1 = 13 * 128, 16 * 128
                    elif st == 12:
                        r0, r1 = 12 * 128, 13 * 128
                    else:
                        continue
                else:
                    r0, r1 = st * 128, (st + 1) * 128
                nc.gpsimd.collective_compute(
                    "ReduceScatter",
                    mybir.AluOpType.add,
                    replica_groups=[[0, 1], [2, 3], [4, 5], [6, 7]],
                    ins=[out_partial[r0:r1, :].opt()],
                    outs=[out_rs[r0 // 2 : r1 // 2, :].opt()],
                )
                nc.sync.dma_start(
                    out=out_ext[r0 // 2 : r1 // 2, :],
                    in_=out_rs[r0 // 2 : r1 // 2, :],
                )

    wo_ctx.__exit__(None, None, None)
    qkres_ctx.__exit__(None, None, None)
    vs_ctx.__exit__(None, None, None)

# ---------------------------------------------------------------------------
# Host entry point
# ---------------------------------------------------------------------------

_NC_CACHE = None


def _get_nc():
    global _NC_CACHE
    if _NC_CACHE is None:
        _NC_CACHE = build_nc()
    return _NC_CACHE


# pair-deinterleave row permutation: within each 128-row head block, even
# rows first then odd rows (matches the rope layout the device expects)
_DEINT = (
    np.arange(NMT)[:, None] * 128
    + np.concatenate([np.arange(0, 128, 2), np.arange(1, 128, 2)])[None, :]
).reshape(-1)


def make_in_maps(
    x, wq, wk, wv, wo,
    lq_down, lq_up, lk_down, lk_up, lv_down, lv_up,
    freqs_cos, freqs_sin, mask,
):
    """Host-side marshalling: fold LoRA into the dense weights (exact),
    shard across cores, and pre-transpose every operand into the layout the
    device consumes directly."""
    f32 = np.float32
    x = np.asarray(x, f32)
    # fold the LoRA low-rank update into the dense weights:
    # x@w.T + ((x@A.T)@B.T)*s  ==  x@(w + s*B@A).T
    wq = np.asarray(wq, f32) + LORA_SCALE * (
        np.asarray(lq_up, f32) @ np.asarray(lq_down, f32))
    wk = np.asarray(wk, f32) + LORA_SCALE * (
        np.asarray(lk_up, f32) @ np.asarray(lk_down, f32))
    wv = np.asarray(wv, f32) + LORA_SCALE * (
        np.asarray(lv_up, f32) @ np.asarray(lv_down, f32))
    wo = np.asarray(wo, f32)
    fcos = np.asarray(freqs_cos, f32)
    fsin = np.asarray(freqs_sin, f32)
    mask = np.asarray(mask, f32)

    import ml_dtypes
    bf16 = ml_dtypes.bfloat16

    c_ = np.ascontiguousarray
    tabC = c_(np.concatenate([fcos.T, fcos.T], axis=0).astype(bf16))  # [128, S]
    tabS = c_(np.concatenate([fsin.T, fsin.T], axis=0).astype(bf16))
    triTs = c_((mask[:128, :128].T * (1.0 / SCALE)).astype(bf16))

    xTs = [c_(x[b].T.astype(bf16)) for b in range(B)]     # [D, S] each

    in_maps = []
    for c in range(N_CORES):
        b, j = c // 2, c % 2
        msl = slice(j * M, (j + 1) * M)
        in_maps.append({
            "xT": xTs[b],
            "wqT": c_(wq[msl, :][_DEINT, :].T.astype(bf16)),
            "wkT": c_(wk[msl, :][_DEINT, :].T.astype(bf16)),
            "wvT": c_(wv[msl, :].T.astype(bf16)),
            "woT": c_(wo[:, msl].T.astype(bf16)),
            "tabC": tabC,
            "tabS": tabS,
            "triTs": triTs,
        })
    return in_maps


def kernel(
    x, wq, wk, wv, wo,
    lq_down, lq_up, lk_down, lk_up, lv_down, lv_up,
    freqs_cos, freqs_sin, mask,
):
    """Full inputs in, full [B, S, D] output out; 8-core SPMD inside."""
    from concourse.bass_utils import run_bass_kernel_spmd

    in_maps = make_in_maps(
        x, wq, wk, wv, wo,
        lq_down, lq_up, lk_down, lk_up, lv_down, lv_up,
        freqs_cos, freqs_sin, mask,
    )
    nc = _get_nc()
    res = run_bass_kernel_spmd(nc, in_maps, list(range(N_CORES)), trace=False)

    return assemble(res.results)


def assemble(results):
    # Pairwise ReduceScatter over row chunks: for chunk [r0, r1), core
    # (2b+j) holds the reduced global rows [r0 + j*n, r0 + (j+1)*n) of
    # batch b at local rows [r0/2, r0/2 + n), n = (r1-r0)/2.  128-row
    # chunks except the last query chunk, which uses two 256-row chunks.
    chunks = [(st * 128, (st + 1) * 128) for st in range(12)]
    chunks += [(1536, 1664), (1664, 2048)]
    out = np.empty((B, S, D), dtype=np.float32)
    for b in range(B):
        for j in range(2):
            buf = np.asarray(results[2 * b + j]["out"], dtype=np.float32)
            for r0, r1 in chunks:
                n = (r1 - r0) // 2
                out[b, r0 + j * n : r0 + (j + 1) * n, :] = \
                    buf[r0 // 2 : r0 // 2 + n, :]
    return out


# revision 23
# speedup vs baseline: 1.1872x; 1.1872x over previous
"""Distributed Trainium2 kernel for LoRA multi-head causal attention.

Problem: out = (softmax(causal(RoPE(x@wq'.T) @ RoPE(x@wk'.T)^T / sqrt(dh)))
               @ (x@wv'.T)) @ wo.T
where wq' = wq + LORA_SCALE * lq_up @ lq_down (LoRA folded into the dense
weights on the HOST -- exact linear algebra, so the device never sees the
low-rank path).
Shapes: B=4, S=2048, D=2048, H=16 heads, DH=128, fp32 I/O.

Sharding (8 NeuronCores): 2-D grid of (batch b = core//2) x (head-group
j = core%2, 8 heads each).  Each core computes its batch's attention for its
8 heads plus the partial o_proj for those heads' feature rows; partials are
summed with a pairwise ReduceScatter (cores 2b, 2b+1), each core emitting
half the sequence rows of batch b.  The host concatenates the halves.

Device-side plan per core (all matmuls bf16 with fp32 PSUM accumulation):
  A. every operand arrives from the HOST pre-transposed (contraction dim
     leading), pair-deinterleaved where RoPE needs it, and pre-cast to
     bf16 -- phase A is pure DMA.
  B. V = x@wv'.T in [s, m] layout, SBUF-resident; then per head Q^T/K^T
     [dh, s] with a fused RoPE eviction writing DIRECTLY into persistent
     SBUF tiles (no DRAM bounce -- Q^T and K^T stay resident through the
     attention phase, 8 MiB).
  C. Attention, query-chunk outer / head inner, causal blocks only:
     scores^T = K_tile @ Q^T (one transposed [128,128] mask tile on the
     diagonal), ACT exp straight out of PSUM into bf16 P^T.  The softmax
     denominator is accumulated on the DVE (acc += P^T tile per key tile)
     and reduced over partitions with ONE ones-column matmul per
     (head, chunk) -- this removes ~90us of [128,1,512] matmuls from the
     in-order PE queue.  P@V accumulates over key tiles in PSUM.
  D. o_proj from the attention outputs' transposed layout; per-head 1/l
     normalization via exp(-ln(l)) on ACT + a ones-broadcast matmul.
  E. Pairwise ReduceScatter of the partial bf16 output per 128-row tile,
     written directly into the kernel output buffer; the collective for
     tile st overlaps o_proj of st+1.
"""

import sys

for _p in ("/opt/trn_rl_repo", "/root/.axon_site/_ro/trn_rl_repo"):
    if _p not in sys.path:
        sys.path.append(_p)

import numpy as np

import concourse.bass as bass
import concourse.mybir as mybir
import concourse.tile as tile
from concourse.masks import make_identity

F32 = mybir.dt.float32
BF16 = mybir.dt.bfloat16
AF = mybir.ActivationFunctionType

B, S, D, H = 4, 2048, 2048, 16
DH = 128
H_LOC = 8           # heads per core
M = H_LOC * DH      # 1024: local qkv feature dim
SCALE = 1.0 / float(np.sqrt(DH))
LORA_SCALE = 32.0 / 16.0
N_CORES = 8
SC = 512            # q/s chunk size
NQC = S // SC       # 4
NDT = D // 128      # 16 contraction tiles
NMT = M // 128      # 8 local m tiles (= local heads)
NST = S // 128      # 16 sequence tiles

# ---------------------------------------------------------------------------
# Walrus in this container rejects instructions carrying more than one sync
# wait ("Too many sync wait commands").  After Tile scheduling, move excess
# semaphore waits onto same-engine nop instructions inserted immediately
# before the over-subscribed instruction (same sequencer, same order, so
# semantics are unchanged).
# ---------------------------------------------------------------------------


def _split_sync_waits(nc, limit=1):
    for bb in nc.main_func.blocks:
        out = []
        changed = False
        for inst in bb.instructions:
            si = inst.sync_info
            waits = list(si.on_wait) if si is not None else []
            if len(waits) > limit:
                changed = True
                extra, keep = waits[:-limit], waits[-limit:]
                for i in range(0, len(extra), limit):
                    n = mybir.InstNoOp(name=nc.get_next_instruction_name())
                    n.engine = inst.engine
                    n.sync_info = mybir.SyncInfo(
                        on_wait=extra[i : i + limit], on_update=[]
                    )
                    nc.register_instruction(n)
                    out.append(n)
                inst.sync_info = mybir.SyncInfo(
                    on_wait=keep, on_update=list(si.on_update)
                )
            out.append(inst)
        if changed:
            bb.instructions = out


class FixedTileContext(tile.TileContext):
    def __exit__(self, exc_type, exc_value, traceback):
        result = super().__exit__(exc_type, exc_value, traceback)
        if exc_type is None:
            _split_sync_waits(self.nc)
        return result


# ---------------------------------------------------------------------------
# Kernel builder (one SPMD graph, identical on all 8 cores)
# ---------------------------------------------------------------------------


def build_nc():
    nc = bass.Bass(target_bir_lowering=False)

    # All weight/activation operands arrive pre-transposed (and where
    # needed pair-deinterleaved) from the host, pre-cast to bf16, with
    # LoRA already folded into the dense weights.
    xT_ext = nc.declare_dram_parameter("xT", [D, S], BF16, isOutput=False)
    wqT_ext = nc.declare_dram_parameter("wqT", [D, M], BF16, isOutput=False)
    wkT_ext = nc.declare_dram_parameter("wkT", [D, M], BF16, isOutput=False)
    wvT_ext = nc.declare_dram_parameter("wvT", [D, M], BF16, isOutput=False)
    woT_ext = nc.declare_dram_parameter("woT", [M, D], BF16, isOutput=False)
    tabC_ext = nc.declare_dram_parameter("tabC", [128, S], BF16, isOutput=False)
    tabS_ext = nc.declare_dram_parameter("tabS", [128, S], BF16, isOutput=False)
    triTs_ext = nc.declare_dram_parameter("triTs", [128, 128], BF16, isOutput=False)
    # bf16 output: the host upcasts to f32 in assemble()
    out_ext = nc.declare_dram_parameter("out", [S // 2, D], BF16, isOutput=True)

    out_partial = nc.dram_tensor("out_partial", [S, D], BF16)
    out_rs = nc.dram_tensor("out_rs", [S // 2, D], BF16)

    with FixedTileContext(nc) as tc:
        with tc.tile_pool(name="const", bufs=1) as const:
            id_bf16 = const.tile([128, 128], BF16, tag="idh")
            make_identity(nc, id_bf16)
            ones_sq = const.tile([128, 128], BF16, tag="ones_sq")
            nc.vector.memset(ones_sq[:, :], 1.0)
            triT_h = const.tile([128, 128], BF16, tag="triT_h")
            # first B2 weight tile, preloaded on the idle scalar queue in
            # phase A: the wtt pool reuses wtv's SBUF zone, so its own DMA
            # loads can't start until B1's last matmul releases it.
            wtb0 = const.tile([128, NDT, 128], BF16, tag="wtb0")

            _phases(nc, tc, locals())
    return nc


def _attn_epilogue(nc, g, sc_ps, linv_pool, at_pool, AT, qc, h, po, acc):
    # Deferred one head so the PE never waits on the DVE accumulation
    # chain's tail.  The all-ones [128,128] stationary replicates the
    # partition sum onto ALL output partitions, so 1/l is one full-rate
    # exact DVE reciprocal -- no ACT chain, no broadcast step.
    pb = sc_ps.tile([128, SC], F32, tag="ps", name=f"pb{h}_{qc}")
    nc.tensor.matmul(
        pb[:, :], g["ones_sq"][:, :], acc[:, :], start=True, stop=True
    )
    # 1/l = exp(-ln(l)) -- two full-partition ACT table ops (~0.4us each at
    # 128 lanes; the exact DVE reciprocal is per-lane serial, ~3.3us)
    lnl = linv_pool.tile([128, SC], F32, tag="lnl", name=f"lnl{h}_{qc}")
    nc.scalar.activation(lnl[:, :], pb[:, :], AF.Ln)
    lb = linv_pool.tile([128, SC], BF16, tag="lb", name=f"lb{h}_{qc}")
    nc.scalar.activation(lb[:, :], lnl[:, :], AF.Exp, scale=-1.0)
    at_ = at_pool.tile([128, SC], BF16, tag=f"aT{h}", name=f"aT{h}_{qc}")
    nc.vector.tensor_mul(at_[:, :], po[:, :], lb[:, :])
    AT[h] = at_


def _phases(nc, tc, t):
    g = t
    xT_ext = g["xT_ext"]
    out_ext, out_partial, out_rs = g["out_ext"], g["out_partial"], g["out_rs"]
    id_bf16 = g["id_bf16"]
    triT_h = g["triT_h"]

    vs_ctx = tc.tile_pool(name="vs", bufs=1)
    vs_pool = vs_ctx.__enter__()
    VS = [vs_pool.tile([128, M], BF16, tag=f"v{st}", name=f"v{st}") for st in range(NST)]
    # Q^T/K^T stay SBUF-resident through the attention phase (8 MiB).
    # Created below ab on the pool stack so they survive ab's release.
    qkres_ctx = tc.tile_pool(name="qkres", bufs=1)
    qkres_pool = qkres_ctx.__enter__()
    QT = [qkres_pool.tile([128, S], BF16, tag=f"qres{h}", name=f"qres{h}")
          for h in range(H_LOC)]
    KT = [qkres_pool.tile([128, S], BF16, tag=f"kres{h}", name=f"kres{h}")
          for h in range(H_LOC)]
    wts_o = {}

    with tc.tile_pool(name="ab", bufs=1) as ab_pool:
        # rope tables live through phase B only.  Layout: cos duplicated on
        # rows 0:64 and 64:128; same for sin (keeps every rope product
        # partition-aligned).  1/sqrt(dh) is folded into the exp activation.
        tabkC = ab_pool.tile([128, S], BF16, tag="tabkC")
        tabkS = ab_pool.tile([128, S], BF16, tag="tabkS")
        XT = [ab_pool.tile([128, S], BF16, tag=f"xT{dt}", name=f"xT{dt}") for dt in range(NDT)]

        wtv_ctx = tc.tile_pool(name="wtv", bufs=16)
        wtv_pool = wtv_ctx.__enter__()
        # ------------------------------------------------------------------
        # Phase A: pure DMA loads.  wv on the gpsimd SWDGE ring, X^T on the
        # sync HWDGE queue in parallel, small constants on the scalar queue.
        # ------------------------------------------------------------------
        wts = [
            wtv_pool.tile([128, M], BF16, tag="wtv", name=f"wvT{dt}")
            for dt in range(NDT)
        ]
        for dt in range(NDT):
            nc.gpsimd.dma_start(
                out=wts[dt][:, :],
                in_=g["wvT_ext"][dt * 128 : (dt + 1) * 128, :],
            )
        for dt in range(NDT):
            nc.sync.dma_start(
                out=XT[dt][:, 0:SC],
                in_=xT_ext[dt * 128 : (dt + 1) * 128, 0:SC],
            )

        nc.scalar.dma_start(out=triT_h[:, :], in_=g["triTs_ext"][:, :])
        nc.scalar.dma_start(out=tabkC[:, :], in_=g["tabC_ext"][:, :])
        nc.scalar.dma_start(out=tabkS[:, :], in_=g["tabS_ext"][:, :])
        rr_q0 = g["wqT_ext"].rearrange("(dt p) m -> p dt m", p=128)
        nc.scalar.dma_start(out=g["wtb0"][:, :, :], in_=rr_q0[:, :, 0:128])

        for dt in range(NDT):
            nc.sync.dma_start(
                out=XT[dt][:, SC : 2 * SC],
                in_=xT_ext[dt * 128 : (dt + 1) * 128, SC : 2 * SC],
            )
        for dt in range(NDT):
            nc.sync.dma_start(
                out=XT[dt][:, 2 * SC : S],
                in_=xT_ext[dt * 128 : (dt + 1) * 128, 2 * SC : S],
            )

        # ------------------------------------------------------------------
        # Phase B1: V projection (SBUF-resident V).  One PSUM pool serves
        # both B1's pv and B2's pq tiles (same shape) so there is no
        # pool-boundary drain between the phases.
        # ------------------------------------------------------------------
        proj_ctx = tc.tile_pool(name="proj_ps", bufs=3, space="PSUM")
        proj_ps1 = proj_ctx.__enter__()
        for st in range(NST):
            pv = proj_ps1.tile([128, M], F32, tag="pv")
            ssl = slice(st * 128, (st + 1) * 128)
            for sub in range(2):
                psl = slice(sub * SC, (sub + 1) * SC)
                for dt in range(NDT):
                    nc.tensor.matmul(
                        pv[:, psl], XT[dt][:, ssl], wts[dt][:, psl],
                        start=(dt == 0), stop=(dt == NDT - 1),
                    )
            nc.vector.tensor_copy(VS[st][:, :], pv[:, :])
        wtv_ctx.__exit__(None, None, None)

        # ------------------------------------------------------------------
        # Phase B2: Q^T / K^T, head outer (q then k per head so attention on
        # head h can begin while head h+1 projects), rope eviction split
        # across ACT (psum spill), DVE, and GpSimd, writing straight into
        # the resident SBUF tiles.
        # ------------------------------------------------------------------
        HS = S // 2  # 1024: half the sequence per psum tile
        proj_ps2 = proj_ps1
        with tc.tile_pool(name="wtt", bufs=3) as wtt_pool, \
             tc.tile_pool(name="rope_tmp", bufs=2) as rope_tmp:
            rr_q = g["wqT_ext"].rearrange("(dt p) m -> p dt m", p=128)
            rr_k = g["wkT_ext"].rearrange("(dt p) m -> p dt m", p=128)
            for mt in range(NMT):
                for which, rr, outd, tabC, tabS in (
                    ("q", rr_q, QT, tabkC, tabkS),
                    ("k", rr_k, KT, tabkC, tabkS),
                ):
                    if mt == 0 and which == "q":
                        wtb = g["wtb0"]  # preloaded in phase A
                    else:
                        wtb = wtt_pool.tile([128, NDT, 128], BF16, tag="wtb")
                        nc.gpsimd.dma_start(
                            out=wtb[:, :, :],
                            in_=rr[:, :, mt * 128 : (mt + 1) * 128],
                        )
                    for half in range(2):
                        sl = slice(half * HS, (half + 1) * HS)
                        pq = proj_ps2.tile([128, HS], F32, tag="pv")
                        for sub in range(2):
                            psl = slice(sub * SC, (sub + 1) * SC)
                            xsl = slice(half * HS + sub * SC, half * HS + (sub + 1) * SC)
                            for dt in range(NDT):
                                nc.tensor.matmul(
                                    pq[:, psl], wtb[:, dt, :], XT[dt][:, xsl],
                                    start=(dt == 0), stop=(dt == NDT - 1),
                                )
                        # rope on deinterleaved pairs: rows 0:64 = a (even
                        # lanes), 64:128 = b (odd lanes); tab rows 0:64 cos,
                        # 64:128 sin.
                        if mt < 6:
                            qsb = rope_tmp.tile([128, HS], BF16, tag="qsb")
                            nc.scalar.copy(qsb[:, :], pq[:, :])
                        t1 = rope_tmp.tile([128, HS], BF16, tag="t1")
                        t2 = rope_tmp.tile([128, HS], BF16, tag="t2")
                        # t1 rows 0:64 = a*cos, rows 64:128 = b*cos.  GpSimd
                        # helps mid-phase; near the phase tail its queue
                        # backlog would delay the final evictions, so the
                        # last heads use DVE instead.
                        if mt < 6:
                            nc.gpsimd.tensor_mul(t1[:, :], qsb[:, :], tabC[:, sl])
                        else:
                            nc.vector.tensor_mul(t1[:, :], pq[:, :], tabC[:, sl])
                        # cross-half products from psum on DVE:
                        # t2 rows 0:64 = b*sin, rows 64:128 = a*sin
                        nc.vector.tensor_mul(t2[0:64, :], pq[64:128, :], tabS[0:64, sl])
                        nc.vector.tensor_mul(t2[64:128, :], pq[0:64, :], tabS[64:128, sl])
                        # even out = a*cos - b*sin ; odd out = a*sin + b*cos
                        # -- written directly into the resident tile
                        nc.vector.tensor_sub(outd[mt][0:64, sl], t1[0:64, :], t2[0:64, :])
                        nc.vector.tensor_add(outd[mt][64:128, sl], t2[64:128, :], t1[64:128, :])

        proj_ctx.__exit__(None, None, None)

    # ----------------------------------------------------------------------
    # Phases C+D interleaved, query-chunk outer: attention for all heads of
    # one 512-query chunk, then that chunk's o_proj rows and its pairwise
    # ReduceScatter -- the collective for chunk c overlaps attention of
    # chunk c+1.
    # ----------------------------------------------------------------------
    wo_ctx = tc.tile_pool(name="wo_t", bufs=8)
    wo_pool = wo_ctx.__enter__()
    with tc.tile_pool(name="at", bufs=2) as at_pool, \
         tc.tile_pool(name="pt", bufs=8) as pt_pool, \
         tc.tile_pool(name="acc", bufs=3) as acc_pool, \
         tc.tile_pool(name="o_sb", bufs=4) as o_sb, \
         tc.tile_pool(name="sc_ps", bufs=4, space="PSUM") as sc_ps, \
         tc.tile_pool(name="pv_ps", bufs=2, space="PSUM") as pv_ps, \
         tc.tile_pool(name="linv", bufs=4) as linv_pool:

        # o_proj weights stream in on the gpsimd ring while attention of
        # chunk 0 runs (first use is chunk 0's o_proj, ~40us in); loading
        # them here keeps them out of phase B's SBUF high-water mark.
        for mt in range(NMT):
            wob = wo_pool.tile([128, D], BF16, tag="wot", name=f"woT{mt}")
            nc.gpsimd.dma_start(
                out=wob[:, :],
                in_=g["woT_ext"][mt * 128 : (mt + 1) * 128, :],
            )
            wts_o[mt] = wob

        # chunk 1 first: its 8x8-key attention gives the o_proj weight DMAs
        # (which can only start once phase B's SBUF zone is released) enough
        # runway before the first o_proj needs them.  Chunk 3 stays last.
        for qc in (1, 0, 2, 3):
            nk = 4 * (qc + 1)
            AT = {}
            pending = []
            for h in range(H_LOC):
                po = pv_ps.tile([128, SC], F32, tag="po")
                acc = acc_pool.tile([128, SC], BF16, tag="acc", name=f"acc{h}_{qc}")
                for ki in range(nk):
                    q_off = max(0, ki * 128 - qc * SC)
                    csl = slice(q_off, SC)
                    ps_ = sc_ps.tile([128, SC], F32, tag="ps")
                    diag = ki * 128 >= qc * SC
                    nc.tensor.matmul(
                        ps_[:, csl],
                        KT[h][:, ki * 128 : (ki + 1) * 128],
                        QT[h][:, qc * SC + q_off : (qc + 1) * SC],
                        start=True, stop=not diag,
                    )
                    if diag:  # causal mask accumulated on the PE itself
                        nc.tensor.matmul(
                            ps_[:, q_off : q_off + 128],
                            id_bf16[:, :],
                            triT_h[:, :],
                            start=False, stop=True,
                        )
                    pt_ = pt_pool.tile([128, SC], BF16, tag="pt")
                    nc.scalar.activation(pt_[:, csl], ps_[:, csl], AF.Exp, scale=SCALE)
                    # softmax denominator: accumulate P^T tiles on the DVE
                    # instead of a per-ki ones-matmul on the in-order PE
                    # queue; the partition reduce happens in the (deferred)
                    # epilogue.
                    if ki == 0:
                        nc.vector.tensor_copy(acc[:, :], pt_[:, :])
                    else:
                        nc.vector.tensor_add(acc[:, csl], acc[:, csl], pt_[:, csl])
                    nc.tensor.matmul(
                        po[:, csl],
                        VS[ki][:, h * 128 : (h + 1) * 128],
                        pt_[:, csl],
                        start=(ki == 0), stop=(ki == nk - 1),
                    )
                pending.append((h, po, acc))
                if len(pending) > 1:
                    _attn_epilogue(nc, g, sc_ps, linv_pool, at_pool, AT, qc, *pending.pop(0))

            while pending:
                _attn_epilogue(nc, g, sc_ps, linv_pool, at_pool, AT, qc, *pending.pop(0))

            # o_proj rows of this chunk; ReduceScatter per 128-row st tile so
            # the collective for st overlaps o_proj of st+1.  For the last
            # chunk, st 12 is processed LAST so the big 384-row collective
            # (rows 13:16) overlaps st 12's o_proj and only a 128-row
            # collective plus a 128-row output copy are exposed at the tail.
            st_order = [13, 14, 15, 12] if qc == NQC - 1 else \
                list(range(qc * 4, (qc + 1) * 4))
            for idx, st in enumerate(st_order):
                ssl = slice((st % 4) * 128, (st % 4 + 1) * 128)
                if idx == 0:
                    # first st of the chunk: interleave the four PSUM
                    # chains -- heads 0-6 for every oc first, head 7 last,
                    # so the last head's epilogue finishes while the PE
                    # works instead of stalling the first chain
                    posums = []
                    for oc in range(NQC):
                        posum = sc_ps.tile(
                            [128, SC], F32, tag="ps", name=f"posum{st}_{oc}"
                        )
                        for mt in range(NMT - 1):
                            nc.tensor.matmul(
                                posum[:, :],
                                AT[mt][:, ssl],
                                wts_o[mt][:, oc * SC : (oc + 1) * SC],
                                start=(mt == 0), stop=False,
                            )
                        posums.append(posum)
                    for oc in range(NQC):
                        nc.tensor.matmul(
                            posums[oc][:, :],
                            AT[NMT - 1][:, ssl],
                            wts_o[NMT - 1][:, oc * SC : (oc + 1) * SC],
                            start=False, stop=True,
                        )
                        ot = o_sb.tile([128, SC], BF16, tag="ot")
                        nc.vector.tensor_copy(ot[:, :], posums[oc][:, :])
                        nc.sync.dma_start(
                            out=out_partial[
                                st * 128 : (st + 1) * 128, oc * SC : (oc + 1) * SC
                            ],
                            in_=ot[:, :],
                        )
                else:
                    for oc in range(NQC):
                        posum = sc_ps.tile(
                            [128, SC], F32, tag="ps", name=f"posum{st}_{oc}"
                        )
                        for mt in range(NMT):
                            nc.tensor.matmul(
                                posum[:, :],
                                AT[mt][:, ssl],
                                wts_o[mt][:, oc * SC : (oc + 1) * SC],
                                start=(mt == 0), stop=(mt == NMT - 1),
                            )
                        ot = o_sb.tile([128, SC], BF16, tag="ot")
                        nc.vector.tensor_copy(ot[:, :], posum[:, :])
                        nc.sync.dma_start(
                            out=out_partial[
                                st * 128 : (st + 1) * 128, oc * SC : (oc + 1) * SC
                            ],
                            in_=ot[:, :],
                        )
                if qc == NQC - 1:
                    if st == 15:
                        # rows 13:16 collective fires while st 12's o_proj
                        # still runs
                        r0, r1 = 13 * 128, 16 * 128
                    elif st == 12:
                        r0, r1 = 12 * 128, 13 * 128
                    else:
                        continue
                else:
                    r0, r1 = st * 128, (st + 1) * 128
                nc.gpsimd.collective_compute(
                    "ReduceScatter",
                    mybir.AluOpType.add,
                    replica_groups=[[0, 1], [2, 3], [4, 5], [6, 7]],
                    ins=[out_partial[r0:r1, :].opt()],
                    outs=[out_rs[r0 // 2 : r1 // 2, :].opt()],
                )
                nc.sync.dma_start(
                    out=out_ext[r0 // 2 : r1 // 2, :],
                    in_=out_rs[r0 // 2 : r1 // 2, :],
                )

    wo_ctx.__exit__(None, None, None)
    qkres_ctx.__exit__(None, None, None)
    vs_ctx.__exit__(None, None, None)

# ---------------------------------------------------------------------------
# Host entry point
# ---------------------------------------------------------------------------

_NC_CACHE = None


def _get_nc():
    global _NC_CACHE
    if _NC_CACHE is None:
        _NC_CACHE = build_nc()
    return _NC_CACHE


# pair-deinterleave row permutation: within each 128-row head block, even
# rows first then odd rows (matches the rope layout the device expects)
_DEINT = (
    np.arange(NMT)[:, None] * 128
    + np.concatenate([np.arange(0, 128, 2), np.arange(1, 128, 2)])[None, :]
).reshape(-1)


def make_in_maps(
    x, wq, wk, wv, wo,
    lq_down, lq_up, lk_down, lk_up, lv_down, lv_up,
    freqs_cos, freqs_sin, mask,
):
    """Host-side marshalling: fold LoRA into the dense weights (exact),
    shard across cores, and pre-transpose every operand into the layout the
    device consumes directly."""
    f32 = np.float32
    x = np.asarray(x, f32)
    # fold the LoRA low-rank update into the dense weights:
    # x@w.T + ((x@A.T)@B.T)*s  ==  x@(w + s*B@A).T
    wq = np.asarray(wq, f32) + LORA_SCALE * (
        np.asarray(lq_up, f32) @ np.asarray(lq_down, f32))
    wk = np.asarray(wk, f32) + LORA_SCALE * (
        np.asarray(lk_up, f32) @ np.asarray(lk_down, f32))
    wv = np.asarray(wv, f32) + LORA_SCALE * (
        np.asarray(lv_up, f32) @ np.asarray(lv_down, f32))
    wo = np.asarray(wo, f32)
    fcos = np.asarray(freqs_cos, f32)
    fsin = np.asarray(freqs_sin, f32)
    mask = np.asarray(mask, f32)

    import ml_dtypes
    bf16 = ml_dtypes.bfloat16

    c_ = np.ascontiguousarray
    tabC = c_(np.concatenate([fcos.T, fcos.T], axis=0).astype(bf16))  # [128, S]
    tabS = c_(np.concatenate([fsin.T, fsin.T], axis=0).astype(bf16))
    triTs = c_((mask[:128, :128].T * (1.0 / SCALE)).astype(bf16))

    xTs = [c_(x[b].T.astype(bf16)) for b in range(B)]     # [D, S] each

    in_maps = []
    for c in range(N_CORES):
        b, j = c // 2, c % 2
        msl = slice(j * M, (j + 1) * M)
        in_maps.append({
            "xT": xTs[b],
            "wqT": c_(wq[msl, :][_DEINT, :].T.astype(bf16)),
            "wkT": c_(wk[msl, :][_DEINT, :].T.astype(bf16)),
            "wvT": c_(wv[msl, :].T.astype(bf16)),
            "woT": c_(wo[:, msl].T.astype(bf16)),
            "tabC": tabC,
            "tabS": tabS,
            "triTs": triTs,
        })
    return in_maps


def kernel(
    x, wq, wk, wv, wo,
    lq_down, lq_up, lk_down, lk_up, lv_down, lv_up,
    freqs_cos, freqs_sin, mask,
):
    """Full inputs in, full [B, S, D] output out; 8-core SPMD inside."""
    from concourse.bass_utils import run_bass_kernel_spmd

    in_maps = make_in_maps(
        x, wq, wk, wv, wo,
        lq_down, lq_up, lk_down, lk_up, lv_down, lv_up,
        freqs_cos, freqs_sin, mask,
    )
    nc = _get_nc()
    res = run_bass_kernel_spmd(nc, in_maps, list(range(N_CORES)), trace=False)

    return assemble(res.results)


def assemble(results):
    # Pairwise ReduceScatter over row chunks: for chunk [r0, r1), core
    # (2b+j) holds the reduced global rows [r0 + j*n, r0 + (j+1)*n) of
    # batch b at local rows [r0/2, r0/2 + n), n = (r1-r0)/2.  128-row
    # chunks except the last query chunk, which uses two 256-row chunks.
    chunks = [(st * 128, (st + 1) * 128) for st in range(12)]
    chunks += [(1536, 1664), (1664, 2048)]
    out = np.empty((B, S, D), dtype=np.float32)
    for b in range(B):
        for j in range(2):
            buf = np.asarray(results[2 * b + j]["out"], dtype=np.float32)
            for r0, r1 in chunks:
                n = (r1 - r0) // 2
                out[b, r0 + j * n : r0 + (j + 1) * n, :] = \
                    buf[r0 // 2 : r0 // 2 + n, :]
    return out


# revision 25
# speedup vs baseline: 1.2000x; 1.0108x over previous
"""Distributed Trainium2 kernel for LoRA multi-head causal attention.

Problem: out = (softmax(causal(RoPE(x@wq'.T) @ RoPE(x@wk'.T)^T / sqrt(dh)))
               @ (x@wv'.T)) @ wo.T
where wq' = wq + LORA_SCALE * lq_up @ lq_down (LoRA folded into the dense
weights on the HOST -- exact linear algebra, so the device never sees the
low-rank path).
Shapes: B=4, S=2048, D=2048, H=16 heads, DH=128, fp32 I/O.

Sharding (8 NeuronCores): 2-D grid of (batch b = core//2) x (head-group
j = core%2, 8 heads each).  Each core computes its batch's attention for its
8 heads plus the partial o_proj for those heads' feature rows; partials are
summed with a pairwise ReduceScatter (cores 2b, 2b+1), each core emitting
half the sequence rows of batch b.  The host concatenates the halves.

Device-side plan per core (all matmuls bf16 with fp32 PSUM accumulation):
  A. every operand arrives from the HOST pre-transposed (contraction dim
     leading), pair-deinterleaved where RoPE needs it, and pre-cast to
     bf16 -- phase A is pure DMA.
  B. V = x@wv'.T in [s, m] layout, SBUF-resident; then per head Q^T/K^T
     [dh, s] with a fused RoPE eviction writing DIRECTLY into persistent
     SBUF tiles (no DRAM bounce -- Q^T and K^T stay resident through the
     attention phase, 8 MiB).
  C. Attention, query-chunk outer / head inner, causal blocks only:
     scores^T = K_tile @ Q^T (one transposed [128,128] mask tile on the
     diagonal), ACT exp straight out of PSUM into bf16 P^T.  The softmax
     denominator is accumulated on the DVE (acc += P^T tile per key tile)
     and reduced over partitions with ONE ones-column matmul per
     (head, chunk) -- this removes ~90us of [128,1,512] matmuls from the
     in-order PE queue.  P@V accumulates over key tiles in PSUM.
  D. o_proj from the attention outputs' transposed layout; per-head 1/l
     normalization via exp(-ln(l)) on ACT + a ones-broadcast matmul.
  E. Pairwise ReduceScatter of the partial bf16 output per 128-row tile,
     written directly into the kernel output buffer; the collective for
     tile st overlaps o_proj of st+1.
"""

import sys

for _p in ("/opt/trn_rl_repo", "/root/.axon_site/_ro/trn_rl_repo"):
    if _p not in sys.path:
        sys.path.append(_p)

import numpy as np

import concourse.bass as bass
import concourse.mybir as mybir
import concourse.tile as tile
from concourse.masks import make_identity

F32 = mybir.dt.float32
BF16 = mybir.dt.bfloat16
AF = mybir.ActivationFunctionType

B, S, D, H = 4, 2048, 2048, 16
DH = 128
H_LOC = 8           # heads per core
M = H_LOC * DH      # 1024: local qkv feature dim
SCALE = 1.0 / float(np.sqrt(DH))
LORA_SCALE = 32.0 / 16.0
N_CORES = 8
SC = 512            # q/s chunk size
NQC = S // SC       # 4
NDT = D // 128      # 16 contraction tiles
NMT = M // 128      # 8 local m tiles (= local heads)
NST = S // 128      # 16 sequence tiles

# ---------------------------------------------------------------------------
# Walrus in this container rejects instructions carrying more than one sync
# wait ("Too many sync wait commands").  After Tile scheduling, move excess
# semaphore waits onto same-engine nop instructions inserted immediately
# before the over-subscribed instruction (same sequencer, same order, so
# semantics are unchanged).
# ---------------------------------------------------------------------------


def _split_sync_waits(nc, limit=1):
    for bb in nc.main_func.blocks:
        out = []
        changed = False
        for inst in bb.instructions:
            si = inst.sync_info
            waits = list(si.on_wait) if si is not None else []
            if len(waits) > limit:
                changed = True
                extra, keep = waits[:-limit], waits[-limit:]
                for i in range(0, len(extra), limit):
                    n = mybir.InstNoOp(name=nc.get_next_instruction_name())
                    n.engine = inst.engine
                    n.sync_info = mybir.SyncInfo(
                        on_wait=extra[i : i + limit], on_update=[]
                    )
                    nc.register_instruction(n)
                    out.append(n)
                inst.sync_info = mybir.SyncInfo(
                    on_wait=keep, on_update=list(si.on_update)
                )
            out.append(inst)
        if changed:
            bb.instructions = out


class FixedTileContext(tile.TileContext):
    def __exit__(self, exc_type, exc_value, traceback):
        result = super().__exit__(exc_type, exc_value, traceback)
        if exc_type is None:
            _split_sync_waits(self.nc)
        return result


# ---------------------------------------------------------------------------
# Kernel builder (one SPMD graph, identical on all 8 cores)
# ---------------------------------------------------------------------------


def build_nc():
    nc = bass.Bass(target_bir_lowering=False)

    # All weight/activation operands arrive pre-transposed (and where
    # needed pair-deinterleaved) from the host, pre-cast to bf16, with
    # LoRA already folded into the dense weights.
    xT_ext = nc.declare_dram_parameter("xT", [D, S], BF16, isOutput=False)
    wqT_ext = nc.declare_dram_parameter("wqT", [D, M], BF16, isOutput=False)
    wkT_ext = nc.declare_dram_parameter("wkT", [D, M], BF16, isOutput=False)
    wvT_ext = nc.declare_dram_parameter("wvT", [D, M], BF16, isOutput=False)
    woT_ext = nc.declare_dram_parameter("woT", [M, D], BF16, isOutput=False)
    tabC_ext = nc.declare_dram_parameter("tabC", [128, S], BF16, isOutput=False)
    tabS_ext = nc.declare_dram_parameter("tabS", [128, S], BF16, isOutput=False)
    triTs_ext = nc.declare_dram_parameter("triTs", [128, 128], BF16, isOutput=False)
    # bf16 output: the host upcasts to f32 in assemble()
    out_ext = nc.declare_dram_parameter("out", [S // 2, D], BF16, isOutput=True)

    out_partial = nc.dram_tensor("out_partial", [S, D], BF16)
    out_rs = nc.dram_tensor("out_rs", [S // 2, D], BF16)

    with FixedTileContext(nc) as tc:
        with tc.tile_pool(name="const", bufs=1) as const:
            id_bf16 = const.tile([128, 128], BF16, tag="idh")
            make_identity(nc, id_bf16)
            ones_sq = const.tile([128, 128], BF16, tag="ones_sq")
            nc.vector.memset(ones_sq[:, :], 1.0)
            triT_h = const.tile([128, 128], BF16, tag="triT_h")
            # first B2 weight tile, preloaded on the idle scalar queue in
            # phase A: the wtt pool reuses wtv's SBUF zone, so its own DMA
            # loads can't start until B1's last matmul releases it.
            wtb0 = const.tile([128, NDT, 128], BF16, tag="wtb0")

            _phases(nc, tc, locals())
    return nc


def _attn_epilogue(nc, g, sc_ps, linv_pool, at_pool, AT, qc, h, po, acc):
    # Deferred one head so the PE never waits on the DVE accumulation
    # chain's tail.  The all-ones [128,128] stationary replicates the
    # partition sum onto ALL output partitions, so 1/l is one full-rate
    # exact DVE reciprocal -- no ACT chain, no broadcast step.
    pb = sc_ps.tile([128, SC], F32, tag="ps", name=f"pb{h}_{qc}")
    nc.tensor.matmul(
        pb[:, :], g["ones_sq"][:, :], acc[:, :], start=True, stop=True
    )
    # 1/l = exp(-ln(l)) -- two full-partition ACT table ops (~0.4us each at
    # 128 lanes; the exact DVE reciprocal is per-lane serial, ~3.3us)
    lnl = linv_pool.tile([128, SC], F32, tag="lnl", name=f"lnl{h}_{qc}")
    nc.scalar.activation(lnl[:, :], pb[:, :], AF.Ln)
    lb = linv_pool.tile([128, SC], BF16, tag="lb", name=f"lb{h}_{qc}")
    nc.scalar.activation(lb[:, :], lnl[:, :], AF.Exp, scale=-1.0)
    at_ = at_pool.tile([128, SC], BF16, tag=f"aT{h}", name=f"aT{h}_{qc}")
    nc.vector.tensor_mul(at_[:, :], po[:, :], lb[:, :])
    AT[h] = at_


def _phases(nc, tc, t):
    g = t
    xT_ext = g["xT_ext"]
    out_ext, out_partial, out_rs = g["out_ext"], g["out_partial"], g["out_rs"]
    id_bf16 = g["id_bf16"]
    triT_h = g["triT_h"]

    vs_ctx = tc.tile_pool(name="vs", bufs=1)
    vs_pool = vs_ctx.__enter__()
    VS = [vs_pool.tile([128, M], BF16, tag=f"v{st}", name=f"v{st}") for st in range(NST)]
    # Q^T/K^T stay SBUF-resident through the attention phase (8 MiB).
    # Created below ab on the pool stack so they survive ab's release.
    qkres_ctx = tc.tile_pool(name="qkres", bufs=1)
    qkres_pool = qkres_ctx.__enter__()
    QT = [qkres_pool.tile([128, S], BF16, tag=f"qres{h}", name=f"qres{h}")
          for h in range(H_LOC)]
    KT = [qkres_pool.tile([128, S], BF16, tag=f"kres{h}", name=f"kres{h}")
          for h in range(H_LOC)]
    wts_o = {}

    with tc.tile_pool(name="ab", bufs=1) as ab_pool:
        # rope tables live through phase B only.  Layout: cos duplicated on
        # rows 0:64 and 64:128; same for sin (keeps every rope product
        # partition-aligned).  1/sqrt(dh) is folded into the exp activation.
        tabkC = ab_pool.tile([128, S], BF16, tag="tabkC")
        tabkS = ab_pool.tile([128, S], BF16, tag="tabkS")
        XT = [ab_pool.tile([128, S], BF16, tag=f"xT{dt}", name=f"xT{dt}") for dt in range(NDT)]

        wtv_ctx = tc.tile_pool(name="wtv", bufs=16)
        wtv_pool = wtv_ctx.__enter__()
        # ------------------------------------------------------------------
        # Phase A: pure DMA loads.  wv on the gpsimd SWDGE ring, X^T on the
        # sync HWDGE queue in parallel, small constants on the scalar queue.
        # ------------------------------------------------------------------
        wts = [
            wtv_pool.tile([128, M], BF16, tag="wtv", name=f"wvT{dt}")
            for dt in range(NDT)
        ]
        for dt in range(NDT):
            nc.gpsimd.dma_start(
                out=wts[dt][:, :],
                in_=g["wvT_ext"][dt * 128 : (dt + 1) * 128, :],
            )
        for dt in range(NDT):
            nc.sync.dma_start(
                out=XT[dt][:, 0:SC],
                in_=xT_ext[dt * 128 : (dt + 1) * 128, 0:SC],
            )

        nc.scalar.dma_start(out=triT_h[:, :], in_=g["triTs_ext"][:, :])
        nc.scalar.dma_start(out=tabkC[:, :], in_=g["tabC_ext"][:, :])
        nc.scalar.dma_start(out=tabkS[:, :], in_=g["tabS_ext"][:, :])
        rr_q0 = g["wqT_ext"].rearrange("(dt p) m -> p dt m", p=128)
        nc.scalar.dma_start(out=g["wtb0"][:, :, :], in_=rr_q0[:, :, 0:128])

        for dt in range(NDT):
            nc.sync.dma_start(
                out=XT[dt][:, SC : 2 * SC],
                in_=xT_ext[dt * 128 : (dt + 1) * 128, SC : 2 * SC],
            )
        for dt in range(NDT):
            nc.sync.dma_start(
                out=XT[dt][:, 2 * SC : S],
                in_=xT_ext[dt * 128 : (dt + 1) * 128, 2 * SC : S],
            )

        # ------------------------------------------------------------------
        # Phase B1: V projection (SBUF-resident V).  One PSUM pool serves
        # both B1's pv and B2's pq tiles (same shape) so there is no
        # pool-boundary drain between the phases.
        # ------------------------------------------------------------------
        proj_ctx = tc.tile_pool(name="proj_ps", bufs=3, space="PSUM")
        proj_ps1 = proj_ctx.__enter__()
        for st in range(NST):
            pv = proj_ps1.tile([128, M], F32, tag="pv")
            ssl = slice(st * 128, (st + 1) * 128)
            for sub in range(2):
                psl = slice(sub * SC, (sub + 1) * SC)
                for dt in range(NDT):
                    nc.tensor.matmul(
                        pv[:, psl], XT[dt][:, ssl], wts[dt][:, psl],
                        start=(dt == 0), stop=(dt == NDT - 1),
                    )
            nc.vector.tensor_copy(VS[st][:, :], pv[:, :])
        wtv_ctx.__exit__(None, None, None)

        # ------------------------------------------------------------------
        # Phase B2: Q^T / K^T, head outer (q then k per head so attention on
        # head h can begin while head h+1 projects), rope eviction split
        # across ACT (psum spill), DVE, and GpSimd, writing straight into
        # the resident SBUF tiles.
        # ------------------------------------------------------------------
        HS = S // 2  # 1024: half the sequence per psum tile
        proj_ps2 = proj_ps1
        with tc.tile_pool(name="wtt", bufs=3) as wtt_pool, \
             tc.tile_pool(name="rope_tmp", bufs=2) as rope_tmp:
            rr_q = g["wqT_ext"].rearrange("(dt p) m -> p dt m", p=128)
            rr_k = g["wkT_ext"].rearrange("(dt p) m -> p dt m", p=128)
            for mt in range(NMT):
                for which, rr, outd, tabC, tabS in (
                    ("q", rr_q, QT, tabkC, tabkS),
                    ("k", rr_k, KT, tabkC, tabkS),
                ):
                    if mt == 0 and which == "q":
                        wtb = g["wtb0"]  # preloaded in phase A
                    else:
                        wtb = wtt_pool.tile([128, NDT, 128], BF16, tag="wtb")
                        nc.gpsimd.dma_start(
                            out=wtb[:, :, :],
                            in_=rr[:, :, mt * 128 : (mt + 1) * 128],
                        )
                    for half in range(2):
                        sl = slice(half * HS, (half + 1) * HS)
                        pq = proj_ps2.tile([128, HS], F32, tag="pv")
                        for sub in range(2):
                            psl = slice(sub * SC, (sub + 1) * SC)
                            xsl = slice(half * HS + sub * SC, half * HS + (sub + 1) * SC)
                            for dt in range(NDT):
                                nc.tensor.matmul(
                                    pq[:, psl], wtb[:, dt, :], XT[dt][:, xsl],
                                    start=(dt == 0), stop=(dt == NDT - 1),
                                )
                        # rope on deinterleaved pairs: rows 0:64 = a (even
                        # lanes), 64:128 = b (odd lanes); tab rows 0:64 cos,
                        # 64:128 sin.
                        if mt < 6:
                            qsb = rope_tmp.tile([128, HS], BF16, tag="qsb")
                            nc.scalar.copy(qsb[:, :], pq[:, :])
                        t1 = rope_tmp.tile([128, HS], BF16, tag="t1")
                        t2 = rope_tmp.tile([128, HS], BF16, tag="t2")
                        # t1 rows 0:64 = a*cos, rows 64:128 = b*cos.  GpSimd
                        # helps mid-phase; near the phase tail its queue
                        # backlog would delay the final evictions, so the
                        # last heads use DVE instead.
                        if mt < 6:
                            nc.gpsimd.tensor_mul(t1[:, :], qsb[:, :], tabC[:, sl])
                        else:
                            nc.vector.tensor_mul(t1[:, :], pq[:, :], tabC[:, sl])
                        # cross-half products from psum on DVE:
                        # t2 rows 0:64 = b*sin, rows 64:128 = a*sin
                        nc.vector.tensor_mul(t2[0:64, :], pq[64:128, :], tabS[0:64, sl])
                        nc.vector.tensor_mul(t2[64:128, :], pq[0:64, :], tabS[64:128, sl])
                        # even out = a*cos - b*sin ; odd out = a*sin + b*cos
                        # -- written directly into the resident tile
                        nc.vector.tensor_sub(outd[mt][0:64, sl], t1[0:64, :], t2[0:64, :])
                        nc.vector.tensor_add(outd[mt][64:128, sl], t2[64:128, :], t1[64:128, :])

        proj_ctx.__exit__(None, None, None)

    # ----------------------------------------------------------------------
    # Phases C+D interleaved, query-chunk outer: attention for all heads of
    # one 512-query chunk, then that chunk's o_proj rows and its pairwise
    # ReduceScatter -- the collective for chunk c overlaps attention of
    # chunk c+1.
    # ----------------------------------------------------------------------
    wo_ctx = tc.tile_pool(name="wo_t", bufs=8)
    wo_pool = wo_ctx.__enter__()
    with tc.tile_pool(name="at", bufs=2) as at_pool, \
         tc.tile_pool(name="pt", bufs=8) as pt_pool, \
         tc.tile_pool(name="acc", bufs=3) as acc_pool, \
         tc.tile_pool(name="o_sb", bufs=4) as o_sb, \
         tc.tile_pool(name="sc_ps", bufs=4, space="PSUM") as sc_ps, \
         tc.tile_pool(name="pv_ps", bufs=2, space="PSUM") as pv_ps, \
         tc.tile_pool(name="linv", bufs=4) as linv_pool:

        # o_proj weights stream in on the gpsimd ring while attention of
        # chunk 0 runs (first use is chunk 0's o_proj, ~40us in); loading
        # them here keeps them out of phase B's SBUF high-water mark.
        for mt in range(NMT):
            wob = wo_pool.tile([128, D], BF16, tag="wot", name=f"woT{mt}")
            nc.gpsimd.dma_start(
                out=wob[:, :],
                in_=g["woT_ext"][mt * 128 : (mt + 1) * 128, :],
            )
            wts_o[mt] = wob

        # chunk 1 first: its 8x8-key attention gives the o_proj weight DMAs
        # (which can only start once phase B's SBUF zone is released) enough
        # runway before the first o_proj needs them.  Chunk 3 stays last.
        for qc in (1, 0, 2, 3):
            nk = 4 * (qc + 1)
            AT = {}
            pending = []
            for h in range(H_LOC):
                po = pv_ps.tile([128, SC], F32, tag="po")
                acc = acc_pool.tile([128, SC], BF16, tag="acc", name=f"acc{h}_{qc}")
                for ki in range(nk):
                    q_off = max(0, ki * 128 - qc * SC)
                    csl = slice(q_off, SC)
                    ps_ = sc_ps.tile([128, SC], F32, tag="ps")
                    diag = ki * 128 >= qc * SC
                    nc.tensor.matmul(
                        ps_[:, csl],
                        KT[h][:, ki * 128 : (ki + 1) * 128],
                        QT[h][:, qc * SC + q_off : (qc + 1) * SC],
                        start=True, stop=not diag,
                    )
                    if diag:  # causal mask accumulated on the PE itself
                        nc.tensor.matmul(
                            ps_[:, q_off : q_off + 128],
                            id_bf16[:, :],
                            triT_h[:, :],
                            start=False, stop=True,
                        )
                    pt_ = pt_pool.tile([128, SC], BF16, tag="pt")
                    nc.scalar.activation(pt_[:, csl], ps_[:, csl], AF.Exp, scale=SCALE)
                    # softmax denominator: accumulate P^T tiles on the DVE
                    # instead of a per-ki ones-matmul on the in-order PE
                    # queue; the partition reduce happens in the (deferred)
                    # epilogue.
                    if ki == 0:
                        nc.vector.tensor_copy(acc[:, :], pt_[:, :])
                    else:
                        nc.vector.tensor_add(acc[:, csl], acc[:, csl], pt_[:, csl])
                    nc.tensor.matmul(
                        po[:, csl],
                        VS[ki][:, h * 128 : (h + 1) * 128],
                        pt_[:, csl],
                        start=(ki == 0), stop=(ki == nk - 1),
                    )
                pending.append((h, po, acc))
                if len(pending) > 1:
                    _attn_epilogue(nc, g, sc_ps, linv_pool, at_pool, AT, qc, *pending.pop(0))

            while pending:
                _attn_epilogue(nc, g, sc_ps, linv_pool, at_pool, AT, qc, *pending.pop(0))

            # o_proj rows of this chunk; ReduceScatter per 128-row st tile so
            # the collective for st overlaps o_proj of st+1.  For the last
            # chunk, st 12 is processed LAST so the big 384-row collective
            # (rows 13:16) overlaps st 12's o_proj and only a 128-row
            # collective plus a 128-row output copy are exposed at the tail.
            st_order = [13, 14, 15, 12] if qc == NQC - 1 else \
                list(range(qc * 4, (qc + 1) * 4))
            for idx, st in enumerate(st_order):
                ssl = slice((st % 4) * 128, (st % 4 + 1) * 128)
                if idx == 0:
                    # first st of the chunk: interleave the four PSUM
                    # chains -- heads 0-6 for every oc first, head 7 last,
                    # so the last head's epilogue finishes while the PE
                    # works instead of stalling the first chain
                    posums = []
                    for oc in range(NQC):
                        posum = sc_ps.tile(
                            [128, SC], F32, tag="ps", name=f"posum{st}_{oc}"
                        )
                        for mt in range(NMT - 1):
                            nc.tensor.matmul(
                                posum[:, :],
                                AT[mt][:, ssl],
                                wts_o[mt][:, oc * SC : (oc + 1) * SC],
                                start=(mt == 0), stop=False,
                            )
                        posums.append(posum)
                    for oc in range(NQC):
                        nc.tensor.matmul(
                            posums[oc][:, :],
                            AT[NMT - 1][:, ssl],
                            wts_o[NMT - 1][:, oc * SC : (oc + 1) * SC],
                            start=False, stop=True,
                        )
                        ot = o_sb.tile([128, SC], BF16, tag="ot")
                        nc.vector.tensor_copy(ot[:, :], posums[oc][:, :])
                        nc.sync.dma_start(
                            out=out_partial[
                                st * 128 : (st + 1) * 128, oc * SC : (oc + 1) * SC
                            ],
                            in_=ot[:, :],
                        )
                else:
                    for oc in range(NQC):
                        posum = sc_ps.tile(
                            [128, SC], F32, tag="ps", name=f"posum{st}_{oc}"
                        )
                        for mt in range(NMT):
                            nc.tensor.matmul(
                                posum[:, :],
                                AT[mt][:, ssl],
                                wts_o[mt][:, oc * SC : (oc + 1) * SC],
                                start=(mt == 0), stop=(mt == NMT - 1),
                            )
                        ot = o_sb.tile([128, SC], BF16, tag="ot")
                        nc.vector.tensor_copy(ot[:, :], posum[:, :])
                        nc.sync.dma_start(
                            out=out_partial[
                                st * 128 : (st + 1) * 128, oc * SC : (oc + 1) * SC
                            ],
                            in_=ot[:, :],
                        )
                # One collective per chunk (the ~15us fixed setup dominates
                # small ReduceScatters, and queue backlog behind many small
                # ones stalls the output path).  The out_ext copy rides the
                # gpsimd queue right behind its RS trigger: the collective
                # engine is serial anyway, so blocking that queue on RS
                # completion delays nothing else.
                if qc == NQC - 1:
                    if st == 15:
                        # rows 13:16 collective fires while st 12's o_proj
                        # still runs; only the 128-row rows-12 collective is
                        # exposed at the tail
                        r0, r1 = 13 * 128, 16 * 128
                    elif st == 12:
                        r0, r1 = 12 * 128, 13 * 128
                    else:
                        continue
                else:
                    if st != st_order[-1]:
                        continue
                    r0, r1 = qc * SC, (qc + 1) * SC
                nc.gpsimd.collective_compute(
                    "ReduceScatter",
                    mybir.AluOpType.add,
                    replica_groups=[[0, 1], [2, 3], [4, 5], [6, 7]],
                    ins=[out_partial[r0:r1, :].opt()],
                    outs=[out_rs[r0 // 2 : r1 // 2, :].opt()],
                )
                nc.gpsimd.dma_start(
                    out=out_ext[r0 // 2 : r1 // 2, :],
                    in_=out_rs[r0 // 2 : r1 // 2, :],
                )

    wo_ctx.__exit__(None, None, None)
    qkres_ctx.__exit__(None, None, None)
    vs_ctx.__exit__(None, None, None)

# ---------------------------------------------------------------------------
# Host entry point
# ---------------------------------------------------------------------------

_NC_CACHE = None


def _get_nc():
    global _NC_CACHE
    if _NC_CACHE is None:
        _NC_CACHE = build_nc()
    return _NC_CACHE


# pair-deinterleave row permutation: within each 128-row head block, even
# rows first then odd rows (matches the rope layout the device expects)
_DEINT = (
    np.arange(NMT)[:, None] * 128
    + np.concatenate([np.arange(0, 128, 2), np.arange(1, 128, 2)])[None, :]
).reshape(-1)


def make_in_maps(
    x, wq, wk, wv, wo,
    lq_down, lq_up, lk_down, lk_up, lv_down, lv_up,
    freqs_cos, freqs_sin, mask,
):
    """Host-side marshalling: fold LoRA into the dense weights (exact),
    shard across cores, and pre-transpose every operand into the layout the
    device consumes directly."""
    f32 = np.float32
    x = np.asarray(x, f32)
    # fold the LoRA low-rank update into the dense weights:
    # x@w.T + ((x@A.T)@B.T)*s  ==  x@(w + s*B@A).T
    wq = np.asarray(wq, f32) + LORA_SCALE * (
        np.asarray(lq_up, f32) @ np.asarray(lq_down, f32))
    wk = np.asarray(wk, f32) + LORA_SCALE * (
        np.asarray(lk_up, f32) @ np.asarray(lk_down, f32))
    wv = np.asarray(wv, f32) + LORA_SCALE * (
        np.asarray(lv_up, f32) @ np.asarray(lv_down, f32))
    wo = np.asarray(wo, f32)
    fcos = np.asarray(freqs_cos, f32)
    fsin = np.asarray(freqs_sin, f32)
    mask = np.asarray(mask, f32)

    import ml_dtypes
    bf16 = ml_dtypes.bfloat16

    c_ = np.ascontiguousarray
    tabC = c_(np.concatenate([fcos.T, fcos.T], axis=0).astype(bf16))  # [128, S]
    tabS = c_(np.concatenate([fsin.T, fsin.T], axis=0).astype(bf16))
    triTs = c_((mask[:128, :128].T * (1.0 / SCALE)).astype(bf16))

    xTs = [c_(x[b].T.astype(bf16)) for b in range(B)]     # [D, S] each

    in_maps = []
    for c in range(N_CORES):
        b, j = c // 2, c % 2
        msl = slice(j * M, (j + 1) * M)
        in_maps.append({
            "xT": xTs[b],
            "wqT": c_(wq[msl, :][_DEINT, :].T.astype(bf16)),
            "wkT": c_(wk[msl, :][_DEINT, :].T.astype(bf16)),
            "wvT": c_(wv[msl, :].T.astype(bf16)),
            "woT": c_(wo[:, msl].T.astype(bf16)),
            "tabC": tabC,
            "tabS": tabS,
            "triTs": triTs,
        })
    return in_maps


def kernel(
    x, wq, wk, wv, wo,
    lq_down, lq_up, lk_down, lk_up, lv_down, lv_up,
    freqs_cos, freqs_sin, mask,
):
    """Full inputs in, full [B, S, D] output out; 8-core SPMD inside."""
    from concourse.bass_utils import run_bass_kernel_spmd

    in_maps = make_in_maps(
        x, wq, wk, wv, wo,
        lq_down, lq_up, lk_down, lk_up, lv_down, lv_up,
        freqs_cos, freqs_sin, mask,
    )
    nc = _get_nc()
    res = run_bass_kernel_spmd(nc, in_maps, list(range(N_CORES)), trace=False)

    return assemble(res.results)


def assemble(results):
    # Pairwise ReduceScatter over row chunks: for chunk [r0, r1), core
    # (2b+j) holds the reduced global rows [r0 + j*n, r0 + (j+1)*n) of
    # batch b at local rows [r0/2, r0/2 + n), n = (r1-r0)/2.  One 512-row
    # chunk per query chunk, except the last, which splits 384 + 128.
    chunks = [(0, 512), (512, 1024), (1024, 1536), (1664, 2048), (1536, 1664)]
    out = np.empty((B, S, D), dtype=np.float32)
    for b in range(B):
        for j in range(2):
            buf = np.asarray(results[2 * b + j]["out"], dtype=np.float32)
            for r0, r1 in chunks:
                n = (r1 - r0) // 2
                out[b, r0 + j * n : r0 + (j + 1) * n, :] = \
                    buf[r0 // 2 : r0 // 2 + n, :]
    return out
